# revision 16
# baseline (speedup 1.0000x reference)
"""BitLinear (ternary-weight quantized linear) Trainium2 kernel — v8.

Math (matches reference):
    delta  = mean(|W|) + 1e-5                    (global scalar)
    Wq     = clip(round(W/delta), -1, 1)         (ternary {-1,0,1})
    gamma  = max(|x|, axis=-1) + 1e-5            (per token row)
    k      = round(127*x/gamma)                  (integers in [-127,127])
    out    = (k @ Wq.T) * delta/127

v8 key facts (all HW-measured this session):
  * bf16 matmul streams 1 col/cycle: N=512 MM = ~220 ns.  fp8e4 DoubleRow
    contracts 256 planes per 512-col MM at ~251 ns -> 1.75x per plane.
  * Wq is exact in fp8e4.  k is NOT (8 bits): fp32->fp8 RNE cast (verified
    bit-exact vs numpy emulation on ACT, DVE and the full transpose chain)
    loses |e|<=4 on |k|>16.  On the fixed test input this costs exactly
    rel 1.84e-2 < 2e-2 (computed in f64 host-side).  The `KB` knob moves
    the first KB k-tiles to exact bf16 MMs to buy margin at ~3%/tile cost.
  * delta stays host-combined in f64 across two launches: a device fp32
    tree costs ~5e-6 on delta -> up to ~8e-3 output error, which would eat
    the fp8 margin.
  * One dma_start on one queue sustains only ~240 GB/s (per-descriptor
    overhead); W streams on two HWDGE queues (sync+scalar) in 2 MB chunks,
    x on the vector queue, out-stores on gpsimd (SWDGE).
  * x is quantized in natural layout (row max = free-axis reduce), cast to
    bf16 k-values, PE-transposed (bf16, 1 cyc/row), and the PSUM evac cast
    writes fp8 pairs directly in the DoubleRow [Ki, Ko=2, M] layout.

Sharding: data-parallel over the 8192 token rows (1024 rows/core); W^T
(layout [i, o], pre-transposed host-side) is replicated.  Launch 1 computes
per-core partial abs-sums of W (1/8 slice each); the host combines in f64.
"""

import numpy as np
from contextlib import ExitStack

import concourse.bass as bass
import concourse.bacc as bacc
import concourse.tile as tile
import concourse.mybir as mybir
from concourse import masks
from concourse.bass_utils import run_bass_kernel_spmd

FP32 = mybir.dt.float32
BF16 = mybir.dt.bfloat16
FP8 = mybir.dt.float8e4
ALU = mybir.AluOpType
AF = mybir.ActivationFunctionType
AX = mybir.AxisListType

N_CORES = 8
B, S, I = 4, 2048, 4096
O = 4096
R = B * S                    # 8192 token rows
RS = R // N_CORES            # 1024 rows per core
EPS = 1e-5
MAGIC = 12582912.0           # 1.5 * 2**23: fp32 round-to-nearest-even trick
KT = I // 128                # 32 contraction k-tiles
TP = KT // 2                 # 16 DoubleRow k-tile pairs
MT = RS // 128               # 8 row tiles per core
NT = O // 512                # 8 output-column blocks
W_SLICE = I // N_CORES       # 512 W^T rows per core for the delta pass

# KB = number of leading k-tiles computed in exact bf16 (error-margin knob).
# KB=0: pure fp8 (rel ~1.84e-2); KB=8: rel ~1.59e-2; KB=32: exact.
# Must be a multiple of 4 (phase-X evac batches) unless 32.
# KB=8 trades ~8% speed for a 28% error margin (robust even to input
# re-draws; the 2e-2 gate is a max-statistic over 33.5M elements).
KB = 8


def _new_nc(enable_asserts=True):
    return bacc.Bacc(
        "TRN2",
        target_bir_lowering=False,
        debug=False,
        enable_asserts=enable_asserts,
        num_devices=N_CORES,
    )


def build_delta_nc(repeat=1, loop_n=1):
    """Per-core partial abs-sums over a [512, 4096] slice of W.

    Emits the raw [128, 128] grid of 128-element chunk sums; the host does
    the remaining reduction in float64 (the all-reduce step).
    """
    nc = _new_nc(enable_asserts=(loop_n == 1))
    ws = nc.dram_tensor("ws", [W_SLICE, I], FP32, kind="ExternalInput").ap()
    partial = nc.dram_tensor("partial", [128, 128], FP32, kind="ExternalOutput").ap()

    with tile.TileContext(nc) as tc, ExitStack() as ctx:
        pool = ctx.enter_context(tc.tile_pool(name="ld", bufs=2))
        spool = ctx.enter_context(tc.tile_pool(name="st", bufs=1))

        def body():
            acc = spool.tile([128, 128], FP32, tag="acc")
            for t in range(W_SLICE // 128):
                wtl = pool.tile([128, I], FP32, tag="wtl")
                nc.sync.dma_start(wtl, ws[128 * t : 128 * (t + 1), :])
                # chunked abs-sum: [128, 32, 128] --sum over last--> [128, 32]
                nc.vector.tensor_reduce(
                    acc[:, 32 * t : 32 * (t + 1)],
                    wtl.rearrange("p (c k) -> p c k", c=32),
                    axis=AX.X,
                    op=ALU.add,
                    apply_absolute_value=True,
                )
            nc.sync.dma_start(partial, acc)

        if loop_n > 1:
            with tc.For_i(0, loop_n, 1):
                body()
        else:
            for _rep in range(repeat):
                body()
    nc.compile()
    return nc


def build_v8_nc(repeat=1, phases="xm", kb=KB, loop_n=1):
    """Main launch: quantize x shard + W^T, fp8-DoubleRow matmul, scale, store.

    kb leading k-tiles run as exact bf16 matmuls; the remaining (32-kb) as
    fp8 DoubleRow pairs accumulating into the same PSUM chains.
    loop_n>1 wraps the body in a For_i hardware loop (timing builds).
    """
    assert kb % 2 == 0 or kb == KT
    ktp = (KT - kb) // 2         # fp8 k-tile pairs
    nc = _new_nc(enable_asserts=(loop_n == 1))
    xs = nc.dram_tensor("xs", [RS, I], FP32, kind="ExternalInput").ap()
    wt = nc.dram_tensor("wt", [I, O], FP32, kind="ExternalInput").ap()
    dsum = nc.dram_tensor("dsum", [128, 1], FP32, kind="ExternalInput").ap()
    out = nc.dram_tensor("out", [RS, O], FP32, kind="ExternalOutput").ap()

    with tile.TileContext(nc) as tc, ExitStack() as ctx:
        const_pool = ctx.enter_context(tc.tile_pool(name="const", bufs=1))
        xt_pool = ctx.enter_context(tc.tile_pool(name="xt", bufs=1))

        ident = const_pool.tile([128, 128], BF16)
        masks.make_identity(nc, ident)

        dsum_sb = const_pool.tile([128, 1], FP32)
        nc.sync.dma_start(dsum_sb, dsum)
        delta = const_pool.tile([128, 1], FP32)
        nc.vector.tensor_scalar(delta, dsum_sb, 1.0 / (I * O), EPS, ALU.mult, ALU.add)
        inv_delta = const_pool.tile([128, 1], FP32)
        nc.vector.reciprocal(inv_delta, delta)
        d127 = const_pool.tile([128, 1], FP32)
        nc.vector.tensor_scalar_mul(d127, delta, 1.0 / 127.0)

        # resident quantized-transposed activations:
        # fp8 pairs  xt8[p, t, ko, m*128]  (DoubleRow lhsT layout), and
        # bf16 exact xtb[p, kt, m*128] for the first kb k-tiles.
        if ktp:
            xt8_all = xt_pool.tile([128, ktp * 2 * RS], FP8)
            xt8 = xt8_all.rearrange("p (t ko r) -> p t ko r", t=ktp, ko=2)
        if kb:
            xtb_all = xt_pool.tile([128, kb * RS], BF16)
            xtb = xtb_all.rearrange("p (kt r) -> p kt r", kt=kb)

        if "x" not in phases:
            if ktp:
                nc.vector.memset(xt8_all, 1.0)
            if kb:
                nc.vector.memset(xtb_all, 1.0)

        def rep_body():
            # ---- Phase X: load, quantize, PE-transpose the x shard ----
            if "x" in phases:
              with ExitStack() as xctx:
                xpool = xctx.enter_context(tc.tile_pool(name="xload", bufs=2))
                tpool = xctx.enter_context(tc.tile_pool(name="xtmp", bufs=2))
                qpool = xctx.enter_context(tc.tile_pool(name="xq", bufs=2))
                gpool = xctx.enter_context(tc.tile_pool(name="gam", bufs=2))
                tpsum = xctx.enter_context(tc.tile_pool(name="tps", bufs=4, space="PSUM"))

                for m in range(MT):
                    xtl = xpool.tile([128, I], FP32, tag="x")
                    nc.gpsimd.dma_start(xtl, xs[128 * m : 128 * (m + 1), :])
                    gm = gpool.tile([128, 1], FP32, tag="gm")
                    nc.vector.tensor_reduce(
                        gm, xtl, axis=AX.X, op=ALU.max, apply_absolute_value=True
                    )
                    gme = gpool.tile([128, 1], FP32, tag="gme")
                    nc.vector.tensor_scalar_add(gme, gm, EPS)
                    rec = gpool.tile([128, 1], FP32, tag="rec")
                    nc.vector.reciprocal(rec, gme)
                    sc = gpool.tile([128, 1], FP32, tag="sc")
                    nc.vector.tensor_scalar_mul(sc, rec, 127.0)
                    # t1 = x * (127/gamma) + MAGIC   (rounds to nearest even)
                    t1 = tpool.tile([128, I], FP32, tag="t1")
                    nc.vector.tensor_scalar(t1, xtl, sc, MAGIC, ALU.mult, ALU.add)
                    # xq = t1 - MAGIC  -> integer k, exact in bf16
                    xq = qpool.tile([128, I], BF16, tag="xq")
                    nc.scalar.activation(xq, t1, AF.Copy, bias=-MAGIC, scale=1.0)
                    # bf16 PE transpose, 4 k-blocks batched per PSUM bank;
                    # evac casts to fp8 pairs (RNE) / bf16 per the KB split
                    for kq in range(KT // 4):
                        pst = tpsum.tile([128, 512], BF16, tag="pst")
                        for j in range(4):
                            kt = 4 * kq + j
                            nc.tensor.transpose(
                                pst[:, 128 * j : 128 * (j + 1)],
                                xq[:, 128 * kt : 128 * (kt + 1)],
                                ident,
                            )
                        kt0 = 4 * kq
                        if kt0 >= kb:
                            # 4 k-tiles = 2 fp8 pairs
                            dst = xt8[:, (kt0 - kb) // 2 : (kt0 - kb) // 2 + 2,
                                      :, 128 * m : 128 * (m + 1)]
                            nc.scalar.activation(
                                dst,
                                pst.rearrange("p (t ko c) -> p t ko c", t=2, ko=2),
                                AF.Copy, bias=0.0, scale=1.0)
                        else:
                            dst = xtb[:, kt0 : kt0 + 4, 128 * m : 128 * (m + 1)]
                            nc.scalar.activation(
                                dst,
                                pst.rearrange("p (kt c) -> p kt c", kt=4),
                                AF.Copy, bias=0.0, scale=1.0)

            # ---- Phase MM: stream W^T, quantize to ternary, matmul ----
            if "m" in phases:
              with ExitStack() as mctx:
                wpool = mctx.enter_context(tc.tile_pool(name="wload", bufs=3))
                w1pool = mctx.enter_context(tc.tile_pool(name="w1", bufs=2))
                w2pool = mctx.enter_context(tc.tile_pool(name="w2", bufs=2))
                wqpool = mctx.enter_context(tc.tile_pool(name="wq", bufs=2))
                opool = mctx.enter_context(tc.tile_pool(name="ost", bufs=4))
                mpsum = mctx.enter_context(tc.tile_pool(name="mps", bufs=1, space="PSUM"))

                for n in range(NT):
                    # quantized W for this 512-col block: fp8 pair layout
                    # wq8[p, t, ko, 512] plus bf16 wqb[p, kt, 512]
                    if ktp:
                        wq8_all = wqpool.tile([128, ktp * 2 * 512], FP8, tag="wq8")
                        wq8 = wq8_all.rearrange("p (t ko c) -> p t ko c", t=ktp, ko=2)
                    if kb:
                        wqb_all = wqpool.tile([128, kb * 512], BF16, tag="wqb",
                                              bufs=1)
                        wqb = wqb_all.rearrange("p (kt c) -> p kt c", kt=kb)
                    # 2 MB chunks of 8 k-tiles, alternating two HWDGE queues
                    for c in range(4):
                        wtl = wpool.tile([128, 8 * 512], FP32, tag="w")
                        src = wt[1024 * c : 1024 * (c + 1),
                                 512 * n : 512 * (n + 1)].rearrange(
                            "(kt p) o -> p kt o", p=128)
                        eng = nc.sync if c % 2 == 0 else nc.scalar
                        eng.dma_start(wtl.rearrange("p (kt o) -> p kt o", kt=8), src)
                        # r = W/delta + MAGIC  (rounded to int by fp32 math)
                        w1 = w1pool.tile([128, 8 * 512], FP32, tag="w1")
                        nc.vector.tensor_scalar(
                            w1, wtl, inv_delta, MAGIC, ALU.mult, ALU.add)
                        # clip to MAGIC +- 1  (== clip(round(W/delta), -1, 1))
                        w2 = w2pool.tile([128, 8 * 512], FP32, tag="w2")
                        nc.gpsimd.tensor_scalar(
                            w2, w1, MAGIC + 1.0, MAGIC - 1.0, ALU.min, ALU.max)
                        # subtract MAGIC -> ternary, cast to fp8/bf16 blocks
                        kt0 = 8 * c
                        if kt0 >= kb:
                            dst = wq8[:, (kt0 - kb) // 2 : (kt0 - kb) // 2 + 4]
                            nc.scalar.activation(
                                dst, w2.rearrange(
                                    "p (t ko c) -> p t ko c", t=4, ko=2),
                                AF.Copy, bias=-MAGIC, scale=1.0)
                        elif kt0 + 8 <= kb:
                            dst = wqb[:, kt0 : kt0 + 8]
                            nc.scalar.activation(
                                dst, w2.rearrange("p (kt o) -> p kt o", kt=8),
                                AF.Copy, bias=-MAGIC, scale=1.0)
                        else:
                            # chunk straddles the bf16/fp8 boundary
                            nb = kb - kt0
                            w23 = w2.rearrange("p (kt o) -> p kt o", kt=8)
                            nc.scalar.activation(
                                wqb[:, kt0 : kt0 + nb], w23[:, 0:nb],
                                AF.Copy, bias=-MAGIC, scale=1.0)
                            dst = wq8[:, 0 : (8 - nb) // 2]
                            nc.scalar.activation(
                                dst, w2[:, nb * 512 :].rearrange(
                                    "p (t ko c) -> p t ko c", t=(8 - nb) // 2, ko=2),
                                AF.Copy, bias=-MAGIC, scale=1.0)

                    # matmuls: m-groups of 4, ping-pong PSUM bank sets
                    for mg in range(MT // 4):
                        psums = {}
                        for mi in range(4):
                            m = 4 * mg + mi
                            psums[mi] = mpsum.tile(
                                [128, 512], FP32,
                                name=f"ps{mg}_{mi}", tag=f"ps{mg}_{mi}")
                        for kt in range(kb):
                            for mi in range(4):
                                m = 4 * mg + mi
                                nc.tensor.matmul(
                                    psums[mi],
                                    xtb[:, kt, 128 * m : 128 * (m + 1)],
                                    wqb[:, kt],
                                    start=(kt == 0),
                                    stop=False,
                                )
                        for t in range(ktp):
                            for mi in range(4):
                                m = 4 * mg + mi
                                nc.tensor.matmul(
                                    psums[mi],
                                    xt8[:, t, :, 128 * m : 128 * (m + 1)],
                                    wq8[:, t],
                                    start=(kb == 0 and t == 0),
                                    stop=(t == ktp - 1),
                                    perf_mode=mybir.MatmulPerfMode.DoubleRow,
                                )
                        for mi in range(4):
                            m = 4 * mg + mi
                            ob = opool.tile([128, 512], FP32, tag="ob")
                            nc.scalar.activation(
                                ob, psums[mi], AF.Copy, bias=0.0, scale=d127)
                            nc.gpsimd.dma_start(
                                out[128 * m : 128 * (m + 1),
                                    512 * n : 512 * (n + 1)], ob)

        if loop_n > 1:
            with tc.For_i(0, loop_n, 1):
                rep_body()
        else:
            for _rep in range(repeat):
                rep_body()
    nc.compile()
    return nc


_NC_CACHE = {}


def _get_nc(name, repeat=1, phases="xm", kb=KB):
    key = (name, repeat, phases, kb)
    if key not in _NC_CACHE:
        if name == "delta":
            _NC_CACHE[key] = build_delta_nc(repeat=repeat)
        else:
            _NC_CACHE[key] = build_v8_nc(repeat=repeat, phases=phases, kb=kb)
    return _NC_CACHE[key]


def kernel(x: np.ndarray, weight: np.ndarray) -> np.ndarray:
    """Two SPMD launches over 8 NeuronCores.

    Launch 1 computes per-core partial abs-sums of W (1/8 slice each); the
    host combines the 8x128 partials in float64 (the all-reduce step) so
    delta matches the fp32 reference to ~1e-7.  Launch 2 is the v8 body
    with the abs-sum broadcast as an input scalar.
    """
    x = np.asarray(x, dtype=np.float32)
    weight = np.asarray(weight, dtype=np.float32)
    core_ids = list(range(N_CORES))

    # host-side staging: W^T so the contraction dim is DMA-partition-major
    wtT = np.ascontiguousarray(weight.T)

    # ---- launch 1: per-core partial abs-sums over 1/8 of W ----
    nc_d = _get_nc("delta")
    in_maps_d = [
        {"ws": np.ascontiguousarray(wtT[c * W_SLICE : (c + 1) * W_SLICE, :])}
        for c in core_ids
    ]
    res_d = run_bass_kernel_spmd(nc_d, in_maps_d, core_ids)
    S_total = np.float64(0.0)
    for r in res_d.results:
        S_total += r["partial"].astype(np.float64).sum()
    dsum = np.full((128, 1), np.float32(S_total), dtype=np.float32)

    # ---- launch 2: v8 body with delta input ----
    nc = _get_nc("v8")

    xf = np.ascontiguousarray(x.reshape(R, I))
    in_maps = [
        {
            "xs": np.ascontiguousarray(xf[c * RS : (c + 1) * RS, :]),
            "wt": wtT,
            "dsum": dsum,
        }
        for c in core_ids
    ]
    res = run_bass_kernel_spmd(nc, in_maps, core_ids)
    outs = [res.results[c]["out"] for c in core_ids]
    return np.concatenate(outs, axis=0).reshape(B, S, O)


if __name__ == "__main__":
    rng = np.random.default_rng(0)
    x = rng.standard_normal((B, S, I), dtype=np.float32)
    w = rng.standard_normal((O, I), dtype=np.float32)
    out = kernel(x, w)
    print("out shape", out.shape, "mean", out.mean(), "std", out.std())
